# revision 21
# baseline (speedup 1.0000x reference)
"""Distributed Trainium2 Bass kernel for AltAttention (cosine-sim attention with
alibi bias + key padding mask + out projection).

Sharding (8 cores): core c -> batch b = c//4, heads [4*(c%4) .. 4*(c%4)+3].
Each core computes qkv for its 4 heads over its batch's 2048 tokens, runs
attention (scores computed transposed: keys on partitions), then two 8-rank
AllToAll collectives (one per head pair, so the first overlaps the second
pair's compute) redistribute attention outputs so core c holds all 1024
channels for its 512-token chunk, where the output projection (+bias) runs.
Host concatenates the 8 per-core [512, 1024] outputs.

Engine mapping highlights:
 - qkv projections run as float32r (fp32 rounded to 11-bit mantissa; ~2x
   faster than fp32 on the PE); attention scores / PV / output projection run
   bf16 (2 elem/cycle moving-operand streaming).
 - alibi (with the key-padding mask pre-folded in on the host) is stored bf16
   and added to scores by identity-weight matmuls accumulating into the
   scores PSUM - the DVE is not involved.
 - Softmax skips the max-subtraction (logits are bounded ~ +-66), denominators
   come free via a ones-column appended to V; division is deferred past the
   AllToAll and folded into the projection input.
 - All-to-all shard indices are batch-dependent but the SPMD program is
   shared, so senders duplicate blocks into both shard halves and receivers
   select the right half with a per-core 0/1 mask input.
"""

import numpy as np
import ml_dtypes

import concourse.bass as bass
import concourse.mybir as mybir
import concourse.tile as tile
from concourse import bacc
from concourse.bass_utils import run_bass_kernel_spmd

BF = ml_dtypes.bfloat16
F32 = mybir.dt.float32
F32R = mybir.dt.float32r
BF16 = mybir.dt.bfloat16
AF = mybir.ActivationFunctionType
ALU = mybir.AluOpType

B, N, C, H = 2, 2048, 1024, 16
D = C // H
LOG_MAX = float(np.log(1.0 / 0.01))
N_CORES = 8
HPC = 4               # heads per core
NEG_MASK = -60.0
# k-tiles with kt % 4 < DVE_ADD_MOD use a DVE tensor-tensor add for the alibi;
# the rest use TensorE identity-matmul accumulation. 0 -> all on TensorE.
DVE_ADD_MOD = 3

TRACE = False
_NC = None


def _round_fp32r(x):
    u = np.ascontiguousarray(x.astype(np.float32)).view(np.uint32)
    lsb = (u >> 12) & 1
    r = (u + 0x7FF + lsb) & 0xFFFFF000
    return r.view(np.float32)


def _build():
    nc = bacc.Bacc("TRN2", target_bir_lowering=False, debug=False, num_devices=N_CORES)

    xT_e = nc.dram_tensor("xT", [C, N], F32R, kind="ExternalInput")
    wqk_e = nc.dram_tensor("wqk", [C, 512], F32R, kind="ExternalInput")
    wv_e = nc.dram_tensor("wv", [C, 256], F32R, kind="ExternalInput")
    alibi_e = nc.dram_tensor("alibi", [HPC, N, N], BF16, kind="ExternalInput")
    logit_e = nc.dram_tensor("logit", [1, 4], F32, kind="ExternalInput")
    eq4_e = nc.dram_tensor("eq4", [128, 4], F32R, kind="ExternalInput")
    f2_e = nc.dram_tensor("f2", [2, 128], F32R, kind="ExternalInput")
    f16_e = nc.dram_tensor("f16", [2, 8, 1024], F32R, kind="ExternalInput")
    ones1_e = nc.dram_tensor("ones1", [1, 128], F32R, kind="ExternalInput")
    onesb_e = nc.dram_tensor("onesb", [1, 128], BF16, kind="ExternalInput")
    idb_e = nc.dram_tensor("idb", [128, 128], BF16, kind="ExternalInput")
    projw_e = nc.dram_tensor("projw", [C, C], BF16, kind="ExternalInput")
    projb_e = nc.dram_tensor("projb", [1, C], BF16, kind="ExternalInput")
    bsel_e = nc.dram_tensor("bsel", [128, 2], F32, kind="ExternalInput")
    out_e = nc.dram_tensor("out", [512, C], F32, kind="ExternalOutput")

    with tile.TileContext(nc) as tc:
        with (
            tc.tile_pool(name="consts", bufs=1) as cpool,
            tc.tile_pool(name="qn", bufs=4) as qn_pool,
            tc.tile_pool(name="vsb", bufs=1) as v_pool,
            tc.tile_pool(name="dram", bufs=1, space="DRAM") as dram,
        ):
            # ---- consts ----
            logit = cpool.tile([1, 4], F32)
            eq4 = cpool.tile([128, 4], F32R)
            f2 = cpool.tile([2, 128], F32R)
            ones1 = cpool.tile([1, 128], F32R)
            onesb = cpool.tile([1, 128], BF16)
            idb = cpool.tile([128, 128], BF16)
            bsel = cpool.tile([128, 2], F32)
            for t, e in ((logit, logit_e), (eq4, eq4_e), (f2, f2_e),
                         (ones1, ones1_e), (onesb, onesb_e), (idb, idb_e),
                         (bsel, bsel_e)):
                nc.scalar.dma_start(t[:], e.ap())

            v_sb = v_pool.tile([128, 16, 4 * 65], BF16)
            for h in range(HPC):
                nc.vector.memset(v_sb[:, :, h * 65 + 64], 1.0)

            qn_sb = [None] * 4

            # =================== PHASE A: qkv projection + normalize ========
            with (
                tc.tile_pool(name="xw", bufs=1) as xw_pool,
                tc.tile_pool(name="qkT", bufs=1) as qkT_pool,
                tc.tile_pool(name="sq", bufs=1) as sq_pool,
                tc.tile_pool(name="smal", bufs=1) as sm_pool,
                tc.tile_pool(name="psA", bufs=4, space="PSUM") as psA,
                tc.tile_pool(name="psS", bufs=1, space="PSUM") as psS,
                tc.tile_pool(name="psR", bufs=2, space="PSUM") as psR,
            ):
                xT = xw_pool.tile([128, 8, N], F32R)
                wqk = xw_pool.tile([128, 8, 512], F32R)
                wv = xw_pool.tile([128, 8, 256], F32R)
                for kt in range(8):
                    nc.scalar.dma_start(xT[:, kt, :], xT_e.ap()[kt * 128:(kt + 1) * 128, :])
                    nc.scalar.dma_start(wqk[:, kt, :], wqk_e.ap()[kt * 128:(kt + 1) * 128, :])
                    nc.scalar.dma_start(wv[:, kt, :], wv_e.ap()[kt * 128:(kt + 1) * 128, :])

                # scale chain: eqs = eq4 * bcast(exp(-2*min(logit, LOG_MAX)))
                rmin = sm_pool.tile([1, 4], F32)
                nc.vector.tensor_scalar_min(rmin[:], logit[:], LOG_MAX)
                isc2 = sm_pool.tile([1, 4], F32R)
                nc.scalar.activation(isc2[:], rmin[:], AF.Exp, scale=-2.0)
                scp = psR.tile([128, 4], F32, tag="rep")
                nc.tensor.matmul(scp[:], ones1[:], isc2[:], start=True, stop=True)
                scs = sm_pool.tile([128, 4], F32)
                nc.vector.tensor_copy(scs[:], scp[:])
                eqs = sm_pool.tile([128, 4], F32R)
                nc.vector.tensor_tensor(eqs[:], eq4[:].bitcast(F32), scs[:], ALU.mult)

                # mt order (0, 2, 1, 3): heads 0/1 (q tiles mt0, k tiles mt2)
                # finish first so phase B can begin while mt1/mt3 compute.
                for mt in (0, 2, 1, 3):
                    qkT = qkT_pool.tile([128, N], F32R, tag="qkT", name=f"qkT{mt}")
                    sq = sq_pool.tile([128, N], F32R, tag="sq", name=f"sq{mt}")
                    pss = [psA.tile([128, 512], F32, tag="ps512", name=f"qk{mt}{nt}")
                           for nt in range(4)]
                    for kt in range(8):
                        for nt in range(4):
                            nc.tensor.matmul(
                                pss[nt][:], wqk[:, kt, mt * 128:(mt + 1) * 128],
                                xT[:, kt, nt * 512:(nt + 1) * 512],
                                start=(kt == 0), stop=(kt == 7))
                    for nt in range(4):
                        nc.scalar.activation(qkT[:, nt * 512:(nt + 1) * 512], pss[nt][:], AF.Copy)
                        nc.vector.tensor_tensor(sq[:, nt * 512:(nt + 1) * 512],
                                                pss[nt][:],
                                                qkT[:, nt * 512:(nt + 1) * 512].bitcast(F32),
                                                ALU.mult)
                    # sumsq per head (block-diag ones), then 1/sqrt
                    elhs = eqs[:, 2 * mt:2 * mt + 2] if mt < 2 else eq4[:, 0:2]
                    rnorm = sm_pool.tile([2, N], F32R, tag="rnorm", name=f"rn{mt}")
                    for half in range(4):
                        ssp = psS.tile([2, 512], F32, tag="ssp", name=f"ssp{mt}{half}",
                                       bufs=2)
                        nc.tensor.matmul(ssp[:], elhs,
                                         sq[:, half * 512:(half + 1) * 512],
                                         start=True, stop=True)
                        rr = sm_pool.tile([2, 512], F32, tag="rr", name=f"rr{mt}{half}")
                        nc.vector.reciprocal_approx_fast(rr[:], ssp[:])
                        nc.scalar.activation(rnorm[:, half * 512:(half + 1) * 512],
                                             rr[:], AF.Sqrt)
                    # qn = bf16(qkT * rep(rnorm))
                    qn = qn_pool.tile([128, N], BF16, tag="qn", name=f"qn{mt}")
                    qn_sb[mt] = qn
                    for nt in range(4):
                        rep = psR.tile([128, 512], F32, tag="rep", name=f"rep{mt}{nt}")
                        nc.tensor.matmul(rep[:], f2[:], rnorm[:, nt * 512:(nt + 1) * 512],
                                         start=True, stop=True)
                        nc.vector.tensor_tensor(qn[:, nt * 512:(nt + 1) * 512],
                                                qkT[:, nt * 512:(nt + 1) * 512].bitcast(F32),
                                                rep[:], ALU.mult)
                    if mt == 2:
                        # v in natural layout [token, head*65(+ones)]
                        for tt in range(16):
                            vps = psA.tile([128, 256], F32, tag="ps512", name=f"v{tt}")
                            for kt in range(8):
                                nc.tensor.matmul(vps[:], xT[:, kt, tt * 128:(tt + 1) * 128],
                                                 wv[:, kt, :], start=(kt == 0), stop=(kt == 7))
                            nc.vector.tensor_copy(
                                v_sb[:, tt].rearrange("p (h d) -> p h d", h=4)[:, :, 0:64],
                                vps[:].rearrange("p (h d) -> p h d", h=4))

            # =================== PHASE B: attention =========================
            from contextlib import ExitStack as _ES
            _bd = _ES()
            al_pool = _bd.enter_context(tc.tile_pool(name="alibi", bufs=17))
            aun_pool = _bd.enter_context(tc.tile_pool(name="aun", bufs=1))
            dsm_pool0 = _bd.enter_context(tc.tile_pool(name="dsm", bufs=1))
            # two half-size all-to-alls, one per head pair
            a2a_in = [dram.tile([8, 65, 512], BF16, name=f"a2ai{i}") for i in range(4)]
            a2a_out = [dram.tile([8, 65, 512], BF16, name=f"a2ao{i}") for i in range(4)]

            with (
                tc.tile_pool(name="pP", bufs=4) as p_pool,
                tc.tile_pool(name="stage", bufs=2) as st_pool,
                tc.tile_pool(name="psSC", bufs=3, space="PSUM") as psSC,
                tc.tile_pool(name="psOA", bufs=1, space="PSUM") as psOA,
            ):
                f16 = [dsm_pool0.tile([8, 1024], F32R, name=f"f16{i}") for i in range(2)]
                nc.gpsimd.dma_start(f16[0][:], f16_e.ap()[0])
                nc.gpsimd.dma_start(f16[1][:], f16_e.ap()[1])
                projw = aun_pool.tile([128, 8, C], BF16)
                for ct in range(8):
                    nc.gpsimd.dma_start(projw[:, ct, :], projw_e.ap()[ct * 128:(ct + 1) * 128, :])
                projb = dsm_pool0.tile([1, C], BF16)
                nc.gpsimd.dma_start(projb[:], projb_e.ap())
                a_lo = aun_pool.tile([128, 8, 512], BF16)
                a_hi = aun_pool.tile([128, 8, 512], BF16)
                a_un = aun_pool.tile([128, 8, 512], BF16)
                ahs = aun_pool.tile([128, 8, 512], BF16)
                rsp_lo = [dsm_pool0.tile([8, 512], BF16, name=f"rsl{i}") for i in range(2)]
                rsp_hi = [dsm_pool0.tile([8, 512], BF16, name=f"rsh{i}") for i in range(2)]
                rcp_r = [dsm_pool0.tile([8, 512], F32R, name=f"rcpr{i}") for i in range(2)]
                for h in range(HPC):
                    mt_q = h // 2
                    mt_k = 2 + h // 2
                    off = 64 * (h % 2)
                    als = []
                    for kt in range(16):
                        al = al_pool.tile([128, N], BF16, tag="al", name=f"al{h}{kt}")
                        nc.sync.dma_start(al[:], alibi_e.ap()[h, kt * 128:(kt + 1) * 128, :])
                        als.append(al)
                    for qc in range(2):
                        oa = psOA.tile([65, 1024], F32, tag="oa", name=f"oa_{h}_{qc}")

                        def pv(kt, pt):
                            for j in range(2):
                                nc.tensor.matmul(
                                    oa[:, j * 512:(j + 1) * 512],
                                    v_sb[:, kt, h * 65:h * 65 + 65],
                                    pt[:, j * 512:(j + 1) * 512],
                                    start=(kt == 0), stop=(kt == 15))

                        prev = None
                        for kt in range(16):
                            al = als[kt]
                            use_dve = (kt % 4) < DVE_ADD_MOD
                            sc = psSC.tile([128, 1024], F32, tag="sc", name=f"sc{h}{kt}{qc}")
                            for j in range(2):
                                q0 = qc * 1024 + j * 512
                                nc.tensor.matmul(
                                    sc[:, j * 512:(j + 1) * 512],
                                    qn_sb[mt_k][off:off + 64, kt * 128:(kt + 1) * 128],
                                    qn_sb[mt_q][off:off + 64, q0:q0 + 512],
                                    start=True, stop=use_dve)
                            if not use_dve:
                                for j in range(2):
                                    q0 = qc * 1024 + j * 512
                                    nc.tensor.matmul(
                                        sc[:, j * 512:(j + 1) * 512], idb[:],
                                        al[:, q0:q0 + 512], start=False, stop=True)
                            else:
                                nc.vector.tensor_tensor(
                                    sc[:], sc[:], al[:, qc * 1024:(qc + 1) * 1024], ALU.add)
                            # PV of the previous k-tile lands between this
                            # tile's accumulate pair and the next one, keeping
                            # >=2 matmuls between same-bank WAW pairs
                            if prev is not None:
                                pv(kt - 1, prev)
                            p = p_pool.tile([128, 1024], BF16, tag="p", name=f"p{h}{kt}{qc}")
                            nc.scalar.activation(p[:], sc[:], AF.Exp)
                            prev = p
                        pv(15, prev)
                        stg = st_pool.tile([65, 1024], BF16, tag="stg", name=f"st{h}{qc}")
                        nc.vector.tensor_copy(stg[:], oa[:])
                        for j in range(2):
                            g = 2 * qc + j  # token chunk within batch
                            for s in (g, g + 4):  # real dest is b*4+g; send both
                                nc.gpsimd.dma_start(a2a_in[h][s, :, :],
                                                    stg[:, j * 512:(j + 1) * 512])
                    nc.gpsimd.collective_compute(
                        "AllToAll", ALU.bypass,
                        replica_groups=[list(range(N_CORES))],
                        ins=[a2a_in[h].opt()],
                        outs=[a2a_out[h].opt()],
                    )
                    par = h // 2   # parity group: heads 0,1 -> even cts
                    for sdr in range(4):
                        r = (h % 2) * 4 + sdr
                        nc.gpsimd.dma_start(rsp_lo[par][r:r + 1, :],
                                            a2a_out[h][sdr, 64:65, :])
                        nc.gpsimd.dma_start(rsp_hi[par][r:r + 1, :],
                                            a2a_out[h][4 + sdr, 64:65, :])
                    if h % 2 == 1:
                        # both heads of this parity done: assemble + select the
                        # matching channel tiles while the next pair computes
                        for ct in range(par, 8, 2):
                            sdr = ct // 2
                            nc.gpsimd.dma_start(a_lo[0:64, ct, :],
                                                a2a_out[h - 1][sdr, 0:64, :])
                            nc.gpsimd.dma_start(a_lo[64:128, ct, :],
                                                a2a_out[h][sdr, 0:64, :])
                            nc.gpsimd.dma_start(a_hi[0:64, ct, :],
                                                a2a_out[h - 1][4 + sdr, 0:64, :])
                            nc.gpsimd.dma_start(a_hi[64:128, ct, :],
                                                a2a_out[h][4 + sdr, 0:64, :])
                            nc.vector.tensor_scalar(a_un[:, ct, :], a_lo[:, ct, :],
                                                    bsel[:, 0:1], None, ALU.mult)
                            nc.vector.tensor_scalar(ahs[:, ct, :], a_hi[:, ct, :],
                                                    bsel[:, 1:2], None, ALU.mult)
                            nc.vector.tensor_tensor(a_un[:, ct, :], a_un[:, ct, :],
                                                    ahs[:, ct, :], ALU.add)
                        rsb = st_pool.tile([8, 512], BF16, tag="rsb", name=f"rsb{par}")
                        nc.vector.tensor_scalar(rsb[:], rsp_lo[par][:],
                                                bsel[0:8, 0:1], None, ALU.mult)
                        rs2 = st_pool.tile([8, 512], BF16, tag="rs2", name=f"rs2{par}")
                        nc.vector.tensor_scalar(rs2[:], rsp_hi[par][:],
                                                bsel[0:8, 1:2], None, ALU.mult)
                        rsf = st_pool.tile([8, 512], F32, tag="rsf", name=f"rsf{par}")
                        nc.vector.tensor_tensor(rsf[:], rsb[:], rs2[:], ALU.add)
                        rcpf = st_pool.tile([8, 512], F32, tag="rcpf", name=f"rcpf{par}")
                        nc.vector.reciprocal_approx_fast(rcpf[:], rsf[:])
                        nc.vector.tensor_copy(rcp_r[par][:], rcpf[:])

            # =================== PHASE D: normalize + projection ============
            with (
                tc.tile_pool(name="dsm2", bufs=1) as dsm_pool,
                tc.tile_pool(name="psDR", bufs=2, space="PSUM") as psDR,
                tc.tile_pool(name="psDO", bufs=4, space="PSUM") as psDO,
            ):
                a_nm = dsm_pool.tile([128, 8, 512], BF16)
                # even-parity channel tiles were ready after the second
                # collective - their normalize + proj partials overlap the
                # last collective's latency
                for ct in (0, 2, 4, 6, 1, 3, 5, 7):
                    rep = psDR.tile([128, 512], F32, tag="drep", name=f"dr{ct}")
                    nc.tensor.matmul(rep[:], f16[ct % 2][:, ct * 128:(ct + 1) * 128],
                                     rcp_r[ct % 2][:], start=True, stop=True)
                    nc.vector.tensor_tensor(a_nm[:, ct, :], a_un[:, ct, :], rep[:],
                                            ALU.mult)

                o_sb = dsm_pool.tile([128, 4, C], F32)
                for co in range(2):
                    opss = [psDO.tile([128, 512], F32, tag="dout", name=f"do{mt}{co}")
                            for mt in range(4)]
                    for ct in (0, 2, 4, 6, 1, 3, 5, 7):
                        for mt in range(4):
                            nc.tensor.matmul(opss[mt][:],
                                             a_nm[:, ct, mt * 128:(mt + 1) * 128],
                                             projw[:, ct, co * 512:(co + 1) * 512],
                                             start=(ct == 0), stop=False)
                    for mt in range(4):
                        nc.tensor.matmul(opss[mt][:], onesb[:],
                                         projb[:, co * 512:(co + 1) * 512],
                                         start=False, stop=True)
                        nc.scalar.activation(o_sb[:, mt, co * 512:(co + 1) * 512],
                                             opss[mt][:], AF.Copy)
                        nc.sync.dma_start(
                            out_e.ap()[mt * 128:(mt + 1) * 128,
                                       co * 512:(co + 1) * 512],
                            o_sb[:, mt, co * 512:(co + 1) * 512])
            _bd.close()

    nc.compile()
    return nc


def _get_nc():
    global _NC
    if _NC is None:
        _NC = _build()
    return _NC


def kernel(x, padding_mask, alibi_bias, qkv_w, proj_w, proj_b, logit_scale):
    x = np.asarray(x, np.float32)
    padding_mask = np.asarray(padding_mask, bool)
    alibi_bias = np.asarray(alibi_bias, np.float32)
    qkv_w = np.asarray(qkv_w, np.float32)
    proj_w = np.asarray(proj_w, np.float32)
    proj_b = np.asarray(proj_b, np.float32)
    logit_scale = np.asarray(logit_scale, np.float32).reshape(H)

    nc = _get_nc()

    eq4 = np.zeros((128, 4), np.float32)
    for j in range(4):
        eq4[(j % 2) * 64:(j % 2) * 64 + 64, j] = 1.0
    f2 = np.zeros((2, 128), np.float32)
    f2[0, 0:64] = 1.0
    f2[1, 64:128] = 1.0
    f16 = np.zeros((2, 8, 1024), np.float32)
    for col_h in range(16):        # head (within batch) owning cols [64h, 64h+64)
        xx, ss = col_h % 4, col_h // 4
        f16[xx // 2, (xx % 2) * 4 + ss, col_h * 64:(col_h + 1) * 64] = 1.0
    ones1 = np.ones((1, 128), np.float32)
    idb = np.eye(128, dtype=np.float32).astype(BF)
    projw = np.ascontiguousarray(proj_w.T).astype(BF)          # [c_in, c_out]
    projb = proj_b.reshape(1, C).astype(BF)

    in_maps = []
    for c in range(N_CORES):
        b = c // 4
        hs = [4 * (c % 4) + i for i in range(4)]
        xT = _round_fp32r(np.ascontiguousarray(x[b].T))
        wq = np.concatenate([qkv_w[h * D:(h + 1) * D, :] for h in hs], 0)
        wk = np.concatenate([qkv_w[C + h * D:C + (h + 1) * D, :] for h in hs], 0)
        wv = np.concatenate([qkv_w[2 * C + h * D:2 * C + (h + 1) * D, :] for h in hs], 0)
        wqk = _round_fp32r(np.ascontiguousarray(np.concatenate([wq, wk], 0).T))
        wv_t = _round_fp32r(np.ascontiguousarray(wv.T))
        # [h, k, q] with the key-padding mask folded in
        al = alibi_bias[b, hs].transpose(0, 2, 1) + np.where(
            padding_mask[b], NEG_MASK, 0.0).astype(np.float32)[None, :, None]
        al = np.ascontiguousarray(al).astype(BF)
        logit = logit_scale[hs].reshape(1, 4)
        bsel = np.zeros((128, 2), np.float32)
        bsel[:, 0 if b == 0 else 1] = 1.0
        in_maps.append({
            "bsel": bsel,
            "xT": xT, "wqk": wqk, "wv": wv_t, "alibi": al,
            "logit": np.ascontiguousarray(logit),
            "eq4": eq4, "f2": f2, "f16": f16, "ones1": ones1,
            "onesb": ones1.astype(BF), "idb": idb,
            "projw": projw, "projb": projb,
        })

    res = run_bass_kernel_spmd(nc, in_maps, core_ids=list(range(N_CORES)),
                               trace=TRACE)
    if TRACE:
        kernel.last_exec_time_ns = res.exec_time_ns
        kernel.last_results = res

    out = np.empty((B, N, C), np.float32)
    for c in range(N_CORES):
        b = c // 4
        g = c % 4
        out[b, g * 512:(g + 1) * 512, :] = res.results[c]["out"]
    return out


# revision 22
# speedup vs baseline: 1.0208x; 1.0208x over previous
"""Distributed Trainium2 Bass kernel for AltAttention (cosine-sim attention with
alibi bias + key padding mask + out projection).

Sharding (8 cores): core c -> batch b = c//4, heads [4*(c%4) .. 4*(c%4)+3].
Each core computes qkv for its 4 heads over its batch's 2048 tokens, runs
attention (scores computed transposed: keys on partitions), then two 8-rank
AllToAll collectives (one per head pair, so the first overlaps the second
pair's compute) redistribute attention outputs so core c holds all 1024
channels for its 512-token chunk, where the output projection (+bias) runs.
Host concatenates the 8 per-core [512, 1024] outputs.

Engine mapping highlights:
 - qkv projections run as float32r (fp32 rounded to 11-bit mantissa; ~2x
   faster than fp32 on the PE); attention scores / PV / output projection run
   bf16 (2 elem/cycle moving-operand streaming).
 - alibi (with the key-padding mask pre-folded in on the host) is stored bf16
   and added to scores by identity-weight matmuls accumulating into the
   scores PSUM - the DVE is not involved.
 - Softmax skips the max-subtraction (logits are bounded ~ +-66), denominators
   come free via a ones-column appended to V; division is deferred past the
   AllToAll and folded into the projection input.
 - All-to-all shard indices are batch-dependent but the SPMD program is
   shared, so senders duplicate blocks into both shard halves and receivers
   select the right half with a per-core 0/1 mask input.
"""

import numpy as np
import ml_dtypes

import concourse.bass as bass
import concourse.mybir as mybir
import concourse.tile as tile
from concourse import bacc
from concourse.bass_utils import run_bass_kernel_spmd

BF = ml_dtypes.bfloat16
F32 = mybir.dt.float32
F32R = mybir.dt.float32r
BF16 = mybir.dt.bfloat16
AF = mybir.ActivationFunctionType
ALU = mybir.AluOpType

B, N, C, H = 2, 2048, 1024, 16
D = C // H
LOG_MAX = float(np.log(1.0 / 0.01))
N_CORES = 8
HPC = 4               # heads per core
NEG_MASK = -60.0
# k-tiles with kt % 4 < DVE_ADD_MOD use a DVE tensor-tensor add for the alibi;
# the rest use TensorE identity-matmul accumulation. 0 -> all on TensorE.
DVE_ADD_MOD = 3

TRACE = False
_NC = None


def _round_fp32r(x):
    u = np.ascontiguousarray(x.astype(np.float32)).view(np.uint32)
    lsb = (u >> 12) & 1
    r = (u + 0x7FF + lsb) & 0xFFFFF000
    return r.view(np.float32)


def _build():
    nc = bacc.Bacc("TRN2", target_bir_lowering=False, debug=False, num_devices=N_CORES)

    xT_e = nc.dram_tensor("xT", [C, N], F32R, kind="ExternalInput")
    wqk_e = nc.dram_tensor("wqk", [C, 512], F32R, kind="ExternalInput")
    wv_e = nc.dram_tensor("wv", [C, 256], F32R, kind="ExternalInput")
    alibi_e = nc.dram_tensor("alibi", [HPC, N, N], BF16, kind="ExternalInput")
    logit_e = nc.dram_tensor("logit", [1, 4], F32, kind="ExternalInput")
    eq4_e = nc.dram_tensor("eq4", [128, 4], F32R, kind="ExternalInput")
    f2_e = nc.dram_tensor("f2", [2, 128], F32R, kind="ExternalInput")
    f16_e = nc.dram_tensor("f16", [2, 8, 1024], F32R, kind="ExternalInput")
    ones1_e = nc.dram_tensor("ones1", [1, 128], F32R, kind="ExternalInput")
    onesb_e = nc.dram_tensor("onesb", [1, 128], BF16, kind="ExternalInput")
    idb_e = nc.dram_tensor("idb", [128, 128], BF16, kind="ExternalInput")
    projw_e = nc.dram_tensor("projw", [C, C], BF16, kind="ExternalInput")
    projb_e = nc.dram_tensor("projb", [1, C], BF16, kind="ExternalInput")
    bsel_e = nc.dram_tensor("bsel", [128, 2], F32, kind="ExternalInput")
    out_e = nc.dram_tensor("out", [512, C], F32, kind="ExternalOutput")

    with tile.TileContext(nc) as tc:
        with (
            tc.tile_pool(name="consts", bufs=1) as cpool,
            tc.tile_pool(name="qn", bufs=4) as qn_pool,
            tc.tile_pool(name="vsb", bufs=1) as v_pool,
            tc.tile_pool(name="dram", bufs=1, space="DRAM") as dram,
        ):
            # ---- consts ----
            logit = cpool.tile([1, 4], F32)
            eq4 = cpool.tile([128, 4], F32R)
            f2 = cpool.tile([2, 128], F32R)
            ones1 = cpool.tile([1, 128], F32R)
            onesb = cpool.tile([1, 128], BF16)
            idb = cpool.tile([128, 128], BF16)
            bsel = cpool.tile([128, 2], F32)
            for t, e in ((logit, logit_e), (eq4, eq4_e), (f2, f2_e),
                         (ones1, ones1_e), (onesb, onesb_e), (idb, idb_e),
                         (bsel, bsel_e)):
                nc.scalar.dma_start(t[:], e.ap())

            v_sb = v_pool.tile([128, 16, 4 * 65], BF16)
            for h in range(HPC):
                nc.vector.memset(v_sb[:, :, h * 65 + 64], 1.0)

            qn_sb = [None] * 4

            # =================== PHASE A: qkv projection + normalize ========
            with (
                tc.tile_pool(name="xw", bufs=1) as xw_pool,
                tc.tile_pool(name="qkT", bufs=1) as qkT_pool,
                tc.tile_pool(name="sq", bufs=1) as sq_pool,
                tc.tile_pool(name="smal", bufs=1) as sm_pool,
                tc.tile_pool(name="psA", bufs=4, space="PSUM") as psA,
                tc.tile_pool(name="psS", bufs=1, space="PSUM") as psS,
                tc.tile_pool(name="psR", bufs=2, space="PSUM") as psR,
            ):
                xT = xw_pool.tile([128, 8, N], F32R)
                wqk = xw_pool.tile([128, 8, 512], F32R)
                wv = xw_pool.tile([128, 8, 256], F32R)
                for kt in range(8):
                    nc.scalar.dma_start(xT[:, kt, :], xT_e.ap()[kt * 128:(kt + 1) * 128, :])
                    nc.scalar.dma_start(wqk[:, kt, :], wqk_e.ap()[kt * 128:(kt + 1) * 128, :])
                    nc.scalar.dma_start(wv[:, kt, :], wv_e.ap()[kt * 128:(kt + 1) * 128, :])

                # scale chain: eqs = eq4 * bcast(exp(-2*min(logit, LOG_MAX)))
                rmin = sm_pool.tile([1, 4], F32)
                nc.vector.tensor_scalar_min(rmin[:], logit[:], LOG_MAX)
                isc2 = sm_pool.tile([1, 4], F32R)
                nc.scalar.activation(isc2[:], rmin[:], AF.Exp, scale=-2.0)
                scp = psR.tile([128, 4], F32, tag="rep")
                nc.tensor.matmul(scp[:], ones1[:], isc2[:], start=True, stop=True)
                scs = sm_pool.tile([128, 4], F32)
                nc.vector.tensor_copy(scs[:], scp[:])
                eqs = sm_pool.tile([128, 4], F32R)
                nc.vector.tensor_tensor(eqs[:], eq4[:].bitcast(F32), scs[:], ALU.mult)

                # mt order (0, 2, 1, 3): heads 0/1 (q tiles mt0, k tiles mt2)
                # finish first so phase B can begin while mt1/mt3 compute.
                for mt in (0, 2, 1, 3):
                    qkT = qkT_pool.tile([128, N], F32R, tag="qkT", name=f"qkT{mt}")
                    sq = sq_pool.tile([128, N], F32R, tag="sq", name=f"sq{mt}")
                    pss = [psA.tile([128, 512], F32, tag="ps512", name=f"qk{mt}{nt}")
                           for nt in range(4)]
                    for kt in range(8):
                        for nt in range(4):
                            nc.tensor.matmul(
                                pss[nt][:], wqk[:, kt, mt * 128:(mt + 1) * 128],
                                xT[:, kt, nt * 512:(nt + 1) * 512],
                                start=(kt == 0), stop=(kt == 7))
                    for nt in range(4):
                        nc.scalar.activation(qkT[:, nt * 512:(nt + 1) * 512], pss[nt][:], AF.Copy)
                        nc.vector.tensor_tensor(sq[:, nt * 512:(nt + 1) * 512],
                                                pss[nt][:],
                                                qkT[:, nt * 512:(nt + 1) * 512].bitcast(F32),
                                                ALU.mult)
                    # sumsq per head (block-diag ones), then 1/sqrt
                    elhs = eqs[:, 2 * mt:2 * mt + 2] if mt < 2 else eq4[:, 0:2]
                    rnorm = sm_pool.tile([2, N], F32R, tag="rnorm", name=f"rn{mt}")
                    for half in range(2):
                        ssp = psS.tile([2, 1024], F32, tag="ssp", name=f"ssp{mt}{half}")
                        for nt2 in range(2):
                            nt = half * 2 + nt2
                            nc.tensor.matmul(ssp[:, nt2 * 512:(nt2 + 1) * 512], elhs,
                                             sq[:, nt * 512:(nt + 1) * 512],
                                             start=True, stop=True)
                        rr = sm_pool.tile([2, 1024], F32, tag="rr", name=f"rr{mt}{half}")
                        nc.vector.reciprocal_approx_fast(rr[:], ssp[:])
                        nc.scalar.activation(rnorm[:, half * 1024:(half + 1) * 1024],
                                             rr[:], AF.Sqrt)
                    # qn = bf16(qkT * rep(rnorm))
                    qn = qn_pool.tile([128, N], BF16, tag="qn", name=f"qn{mt}")
                    qn_sb[mt] = qn
                    for nt in range(4):
                        rep = psR.tile([128, 512], F32, tag="rep", name=f"rep{mt}{nt}")
                        nc.tensor.matmul(rep[:], f2[:], rnorm[:, nt * 512:(nt + 1) * 512],
                                         start=True, stop=True)
                        nc.vector.tensor_tensor(qn[:, nt * 512:(nt + 1) * 512],
                                                qkT[:, nt * 512:(nt + 1) * 512].bitcast(F32),
                                                rep[:], ALU.mult)
                    if mt == 2:
                        # v in natural layout [token, head*65(+ones)]
                        for tt in range(16):
                            vps = psA.tile([128, 256], F32, tag="ps512", name=f"v{tt}")
                            for kt in range(8):
                                nc.tensor.matmul(vps[:], xT[:, kt, tt * 128:(tt + 1) * 128],
                                                 wv[:, kt, :], start=(kt == 0), stop=(kt == 7))
                            nc.vector.tensor_copy(
                                v_sb[:, tt].rearrange("p (h d) -> p h d", h=4)[:, :, 0:64],
                                vps[:].rearrange("p (h d) -> p h d", h=4))

            # =================== PHASE B: attention =========================
            from contextlib import ExitStack as _ES
            _bd = _ES()
            al_pool = _bd.enter_context(tc.tile_pool(name="alibi", bufs=17))
            aun_pool = _bd.enter_context(tc.tile_pool(name="aun", bufs=1))
            dsm_pool0 = _bd.enter_context(tc.tile_pool(name="dsm", bufs=1))
            # two half-size all-to-alls, one per head pair
            a2a_in = [dram.tile([8, 65, 512], BF16, name=f"a2ai{i}") for i in range(4)]
            a2a_out = [dram.tile([8, 65, 512], BF16, name=f"a2ao{i}") for i in range(4)]

            with (
                tc.tile_pool(name="pP", bufs=4) as p_pool,
                tc.tile_pool(name="stage", bufs=2) as st_pool,
                tc.tile_pool(name="psSC", bufs=3, space="PSUM") as psSC,
                tc.tile_pool(name="psOA", bufs=1, space="PSUM") as psOA,
            ):
                f16 = [dsm_pool0.tile([8, 1024], F32R, name=f"f16{i}") for i in range(2)]
                nc.gpsimd.dma_start(f16[0][:], f16_e.ap()[0])
                nc.gpsimd.dma_start(f16[1][:], f16_e.ap()[1])
                projw = aun_pool.tile([128, 8, C], BF16)
                for ct in range(8):
                    nc.gpsimd.dma_start(projw[:, ct, :], projw_e.ap()[ct * 128:(ct + 1) * 128, :])
                projb = dsm_pool0.tile([1, C], BF16)
                nc.gpsimd.dma_start(projb[:], projb_e.ap())
                a_lo = aun_pool.tile([128, 8, 512], BF16)
                a_hi = aun_pool.tile([128, 8, 512], BF16)
                a_un = aun_pool.tile([128, 8, 512], BF16)
                ahs = aun_pool.tile([128, 8, 512], BF16)
                rsp_lo = [dsm_pool0.tile([8, 512], BF16, name=f"rsl{i}") for i in range(2)]
                rsp_hi = [dsm_pool0.tile([8, 512], BF16, name=f"rsh{i}") for i in range(2)]
                rcp_r = [dsm_pool0.tile([8, 512], F32R, name=f"rcpr{i}") for i in range(2)]
                for h in range(HPC):
                    mt_q = h // 2
                    mt_k = 2 + h // 2
                    off = 64 * (h % 2)
                    als = []
                    for kt in range(16):
                        al = al_pool.tile([128, N], BF16, tag="al", name=f"al{h}{kt}")
                        nc.sync.dma_start(al[:], alibi_e.ap()[h, kt * 128:(kt + 1) * 128, :])
                        als.append(al)
                    for qc in range(2):
                        oa = psOA.tile([65, 1024], F32, tag="oa", name=f"oa_{h}_{qc}")

                        def pv(kt, pt):
                            for j in range(2):
                                nc.tensor.matmul(
                                    oa[:, j * 512:(j + 1) * 512],
                                    v_sb[:, kt, h * 65:h * 65 + 65],
                                    pt[:, j * 512:(j + 1) * 512],
                                    start=(kt == 0), stop=(kt == 15))

                        prev = None
                        for kt in range(16):
                            al = als[kt]
                            use_dve = (kt % 4) < DVE_ADD_MOD
                            sc = psSC.tile([128, 1024], F32, tag="sc", name=f"sc{h}{kt}{qc}")
                            for j in range(2):
                                q0 = qc * 1024 + j * 512
                                nc.tensor.matmul(
                                    sc[:, j * 512:(j + 1) * 512],
                                    qn_sb[mt_k][off:off + 64, kt * 128:(kt + 1) * 128],
                                    qn_sb[mt_q][off:off + 64, q0:q0 + 512],
                                    start=True, stop=use_dve)
                            if not use_dve:
                                for j in range(2):
                                    q0 = qc * 1024 + j * 512
                                    nc.tensor.matmul(
                                        sc[:, j * 512:(j + 1) * 512], idb[:],
                                        al[:, q0:q0 + 512], start=False, stop=True)
                            else:
                                nc.vector.tensor_tensor(
                                    sc[:], sc[:], al[:, qc * 1024:(qc + 1) * 1024], ALU.add)
                            # PV of the previous k-tile lands between this
                            # tile's accumulate pair and the next one, keeping
                            # >=2 matmuls between same-bank WAW pairs
                            if prev is not None:
                                pv(kt - 1, prev)
                            p = p_pool.tile([128, 1024], BF16, tag="p", name=f"p{h}{kt}{qc}")
                            nc.scalar.activation(p[:], sc[:], AF.Exp)
                            prev = p
                        pv(15, prev)
                        stg = st_pool.tile([65, 1024], BF16, tag="stg", name=f"st{h}{qc}")
                        nc.vector.tensor_copy(stg[:], oa[:])
                        for j in range(2):
                            g = 2 * qc + j  # token chunk within batch
                            for s in (g, g + 4):  # real dest is b*4+g; send both
                                nc.gpsimd.dma_start(a2a_in[h][s, :, :],
                                                    stg[:, j * 512:(j + 1) * 512])
                    nc.gpsimd.collective_compute(
                        "AllToAll", ALU.bypass,
                        replica_groups=[list(range(N_CORES))],
                        ins=[a2a_in[h].opt()],
                        outs=[a2a_out[h].opt()],
                    )
                    par = h // 2   # parity group: heads 0,1 -> even cts
                    for sdr in range(4):
                        r = (h % 2) * 4 + sdr
                        nc.gpsimd.dma_start(rsp_lo[par][r:r + 1, :],
                                            a2a_out[h][sdr, 64:65, :])
                        nc.gpsimd.dma_start(rsp_hi[par][r:r + 1, :],
                                            a2a_out[h][4 + sdr, 64:65, :])
                    if h % 2 == 1:
                        # both heads of this parity done: assemble + select the
                        # matching channel tiles while the next pair computes
                        for ct in range(par, 8, 2):
                            sdr = ct // 2
                            nc.gpsimd.dma_start(a_lo[0:64, ct, :],
                                                a2a_out[h - 1][sdr, 0:64, :])
                            nc.gpsimd.dma_start(a_lo[64:128, ct, :],
                                                a2a_out[h][sdr, 0:64, :])
                            nc.gpsimd.dma_start(a_hi[0:64, ct, :],
                                                a2a_out[h - 1][4 + sdr, 0:64, :])
                            nc.gpsimd.dma_start(a_hi[64:128, ct, :],
                                                a2a_out[h][4 + sdr, 0:64, :])
                            nc.vector.tensor_scalar(a_un[:, ct, :], a_lo[:, ct, :],
                                                    bsel[:, 0:1], None, ALU.mult)
                            nc.vector.tensor_scalar(ahs[:, ct, :], a_hi[:, ct, :],
                                                    bsel[:, 1:2], None, ALU.mult)
                            nc.vector.tensor_tensor(a_un[:, ct, :], a_un[:, ct, :],
                                                    ahs[:, ct, :], ALU.add)
                        rsb = st_pool.tile([8, 512], BF16, tag="rsb", name=f"rsb{par}")
                        nc.vector.tensor_scalar(rsb[:], rsp_lo[par][:],
                                                bsel[0:8, 0:1], None, ALU.mult)
                        rs2 = st_pool.tile([8, 512], BF16, tag="rs2", name=f"rs2{par}")
                        nc.vector.tensor_scalar(rs2[:], rsp_hi[par][:],
                                                bsel[0:8, 1:2], None, ALU.mult)
                        rsf = st_pool.tile([8, 512], F32, tag="rsf", name=f"rsf{par}")
                        nc.vector.tensor_tensor(rsf[:], rsb[:], rs2[:], ALU.add)
                        rcpf = st_pool.tile([8, 512], F32, tag="rcpf", name=f"rcpf{par}")
                        nc.vector.reciprocal_approx_fast(rcpf[:], rsf[:])
                        nc.vector.tensor_copy(rcp_r[par][:], rcpf[:])

            # =================== PHASE D: normalize + projection ============
            with (
                tc.tile_pool(name="dsm2", bufs=1) as dsm_pool,
                tc.tile_pool(name="psDR", bufs=2, space="PSUM") as psDR,
                tc.tile_pool(name="psDO", bufs=4, space="PSUM") as psDO,
            ):
                a_nm = dsm_pool.tile([128, 8, 512], BF16)
                # even-parity channel tiles were ready after the second
                # collective - their normalize + proj partials overlap the
                # last collective's latency
                for ct in (0, 2, 4, 6, 1, 3, 5, 7):
                    rep = psDR.tile([128, 512], F32, tag="drep", name=f"dr{ct}")
                    nc.tensor.matmul(rep[:], f16[ct % 2][:, ct * 128:(ct + 1) * 128],
                                     rcp_r[ct % 2][:], start=True, stop=True)
                    nc.vector.tensor_tensor(a_nm[:, ct, :], a_un[:, ct, :], rep[:],
                                            ALU.mult)

                o_sb = dsm_pool.tile([128, 4, C], F32)
                for co in range(2):
                    opss = [psDO.tile([128, 512], F32, tag="dout", name=f"do{mt}{co}")
                            for mt in range(4)]
                    for ct in (0, 2, 4, 6, 1, 3, 5, 7):
                        for mt in range(4):
                            nc.tensor.matmul(opss[mt][:],
                                             a_nm[:, ct, mt * 128:(mt + 1) * 128],
                                             projw[:, ct, co * 512:(co + 1) * 512],
                                             start=(ct == 0), stop=False)
                    for mt in range(4):
                        nc.tensor.matmul(opss[mt][:], onesb[:],
                                         projb[:, co * 512:(co + 1) * 512],
                                         start=False, stop=True)
                        nc.scalar.activation(o_sb[:, mt, co * 512:(co + 1) * 512],
                                             opss[mt][:], AF.Copy)
                        nc.sync.dma_start(
                            out_e.ap()[mt * 128:(mt + 1) * 128,
                                       co * 512:(co + 1) * 512],
                            o_sb[:, mt, co * 512:(co + 1) * 512])
            _bd.close()

    nc.compile()
    return nc


def _get_nc():
    global _NC
    if _NC is None:
        _NC = _build()
    return _NC


def kernel(x, padding_mask, alibi_bias, qkv_w, proj_w, proj_b, logit_scale):
    x = np.asarray(x, np.float32)
    padding_mask = np.asarray(padding_mask, bool)
    alibi_bias = np.asarray(alibi_bias, np.float32)
    qkv_w = np.asarray(qkv_w, np.float32)
    proj_w = np.asarray(proj_w, np.float32)
    proj_b = np.asarray(proj_b, np.float32)
    logit_scale = np.asarray(logit_scale, np.float32).reshape(H)

    nc = _get_nc()

    eq4 = np.zeros((128, 4), np.float32)
    for j in range(4):
        eq4[(j % 2) * 64:(j % 2) * 64 + 64, j] = 1.0
    f2 = np.zeros((2, 128), np.float32)
    f2[0, 0:64] = 1.0
    f2[1, 64:128] = 1.0
    f16 = np.zeros((2, 8, 1024), np.float32)
    for col_h in range(16):        # head (within batch) owning cols [64h, 64h+64)
        xx, ss = col_h % 4, col_h // 4
        f16[xx // 2, (xx % 2) * 4 + ss, col_h * 64:(col_h + 1) * 64] = 1.0
    ones1 = np.ones((1, 128), np.float32)
    idb = np.eye(128, dtype=np.float32).astype(BF)
    projw = np.ascontiguousarray(proj_w.T).astype(BF)          # [c_in, c_out]
    projb = proj_b.reshape(1, C).astype(BF)

    in_maps = []
    for c in range(N_CORES):
        b = c // 4
        hs = [4 * (c % 4) + i for i in range(4)]
        xT = _round_fp32r(np.ascontiguousarray(x[b].T))
        wq = np.concatenate([qkv_w[h * D:(h + 1) * D, :] for h in hs], 0)
        wk = np.concatenate([qkv_w[C + h * D:C + (h + 1) * D, :] for h in hs], 0)
        wv = np.concatenate([qkv_w[2 * C + h * D:2 * C + (h + 1) * D, :] for h in hs], 0)
        wqk = _round_fp32r(np.ascontiguousarray(np.concatenate([wq, wk], 0).T))
        wv_t = _round_fp32r(np.ascontiguousarray(wv.T))
        # [h, k, q] with the key-padding mask folded in
        al = alibi_bias[b, hs].transpose(0, 2, 1) + np.where(
            padding_mask[b], NEG_MASK, 0.0).astype(np.float32)[None, :, None]
        al = np.ascontiguousarray(al).astype(BF)
        logit = logit_scale[hs].reshape(1, 4)
        bsel = np.zeros((128, 2), np.float32)
        bsel[:, 0 if b == 0 else 1] = 1.0
        in_maps.append({
            "bsel": bsel,
            "xT": xT, "wqk": wqk, "wv": wv_t, "alibi": al,
            "logit": np.ascontiguousarray(logit),
            "eq4": eq4, "f2": f2, "f16": f16, "ones1": ones1,
            "onesb": ones1.astype(BF), "idb": idb,
            "projw": projw, "projb": projb,
        })

    res = run_bass_kernel_spmd(nc, in_maps, core_ids=list(range(N_CORES)),
                               trace=TRACE)
    if TRACE:
        kernel.last_exec_time_ns = res.exec_time_ns
        kernel.last_results = res

    out = np.empty((B, N, C), np.float32)
    for c in range(N_CORES):
        b = c // 4
        g = c % 4
        out[b, g * 512:(g + 1) * 512, :] = res.results[c]["out"]
    return out
